# revision 27
# baseline (speedup 1.0000x reference)
"""Trainium2 Bass kernel for nn_NodeGenerator (GNN message passing).

Strategy (8 NeuronCores, SPMD, no collectives):
  - Only candidate nodes (softmax class-0 > 0.5 and deg > 0, ~12% of N)
    produce nonzero output rows. Host computes the mask (f64) and
    COMPACTS: candidates are dealt round-robin (by descending degree)
    to the 8 cores, ~1508/core, padded to 24 windows of 64 owners.
  - The full node-feature table is replicated per core as 4 quartile
    tables of fp16 rows zero-padded to 128 elems (256 B — the dma_gather
    minimum), so int16 indices can address any row and gathered rows
    feed the PE directly with no convert pass.
  - Directed edges of each core's owners are grouped per
    (supergroup of 4 windows, quartile) and fetched with ONE dma_gather
    per group (24 calls/core, ~2.3K rows each; ring cost is
    num_idxs/16+1 so large calls fit the default SWDGE ring).
  - Per 64-owner window: one-hot S [128, T, 64] built on DVE
    (iota-compare vs per-row owner column), then T accumulating fp16
    PE matmuls G_tile.T @ S_tile give feature-major neighbor sums
    [64, 64] in fp32 PSUM.
  - Neighbor mean + MLP run feature-major over 3 chunks of 512
    candidate columns; recip is broadcast via a rank-1 matmul. No mask
    multiply on device: every compacted column is a candidate, and pad
    columns are simply never scattered back on host.
"""

import numpy as np

N = 100000
D = 64
CORES = 8
NQ = 4
VQ = 25000          # rows per quartile table
WC = 64             # owners per window
WIN = 24            # windows per core
CAP = WIN * WC      # 1536 candidate slots per core
SGW = 4             # windows per gather supergroup
NSG = WIN // SGW    # 6 supergroups
CHUNK = SGW * WC    # MLP column tile = one supergroup (256)
NCHUNK = CAP // CHUNK
GATHER_TILES = 8    # tiles (128 rows each) per dma_gather call: the SWDGE
                    # ring holds 1024 row-descriptors (hard cap per call),
                    # and per-call DMA-engine setup (~0.45us) favors max size


def _host_prep(node_features, node_operations, edge_index):
    fp16 = np.float16
    X = np.ascontiguousarray(np.asarray(node_features, dtype=np.float32))
    ops = np.asarray(node_operations, dtype=np.float32)
    ei = np.asarray(edge_index, dtype=np.int64)
    src, dst = ei[0], ei[1]
    U = np.concatenate([src, dst])
    V = np.concatenate([dst, src])

    deg = np.bincount(U, minlength=N).astype(np.int64)
    o = ops.astype(np.float64)
    e = np.exp(o - o.max(axis=1, keepdims=True))
    p0 = e[:, 0] / e.sum(axis=1)
    maskf = (p0 > 0.5) & (deg > 0)
    recip = (1.0 / np.maximum(deg, 1.0)).astype(np.float32)

    # Compact: deal candidates (desc degree) to cores; each consecutive
    # 8-group lands on one slot position. Within a group the core
    # permutation is chosen to balance per-(window, quartile) loads, which
    # sets the shared (max-over-core) gather row caps.
    cand = np.where(maskf)[0]
    cand = cand[np.argsort(-deg[cand], kind="stable")]
    ncand = len(cand)
    candidx = np.full(N, -1, np.int64)
    candidx[cand] = np.arange(ncand)

    keepe = maskf[U]
    Uk, Vk = U[keepe], V[keepe]
    qe = Vk // VQ
    qv = np.zeros((ncand, NQ), np.int64)
    np.add.at(qv, (candidx[Uk], qe), 1)

    core_of_c = np.empty(ncand, np.int64)
    load = np.zeros((CORES, NQ), np.int64)
    for i0 in range(0, ncand, CORES):
        if i0 % (WC * CORES) == 0:
            load[:] = 0
        n = min(CORES, ncand - i0)
        freec = list(range(CORES))
        for i in range(i0, i0 + n):
            cost = ((load[freec] + qv[i]) ** 2).sum(axis=1)
            j = int(np.argmin(cost))
            c = freec.pop(j)
            core_of_c[i] = c
            load[c] += qv[i]
    pos_of_c = np.arange(ncand) // CORES
    assert pos_of_c.max() < CAP, (ncand, CAP)

    core_of_node = np.full(N, -1, np.int64)
    pos_of_node = np.full(N, -1, np.int64)
    core_of_node[cand] = core_of_c
    pos_of_node[cand] = pos_of_c

    # node id per (core, slot) for output scatter
    slot_node = np.full((CORES, CAP), -1, np.int64)
    slot_node[core_of_c, pos_of_c] = cand

    # Edges owned by candidates
    keep = maskf[U]
    Uk, Vk = U[keep], V[keep]
    cc = core_of_node[Uk]
    pp = pos_of_node[Uk]
    w = pp // WC
    col = (pp % WC).astype(np.float32)
    q = Vk // VQ
    vloc = (Vk - q * VQ).astype(np.int16)

    # counts per (core, w, q); shared row caps = max over cores. Windows are
    # packed back-to-back (no per-window tile alignment) inside each
    # (supergroup, quartile) block; only block ends align to 128.
    gk = (cc * WIN + w) * NQ + q
    cnt = np.bincount(gk, minlength=CORES * WIN * NQ).reshape(CORES, WIN, NQ)
    maxcnt = cnt.max(axis=0)                             # [WIN, NQ] rows

    rowstart = np.zeros((WIN, NQ), np.int64)
    run = 0                                              # rows
    sg_base = np.zeros(NSG, np.int64)                    # tiles
    sg_tiles = np.zeros(NSG, np.int64)
    call_info = []   # (sg, q, tile_base_global, ntiles)
    for sg in range(NSG):
        sg_base[sg] = run // 128
        for qq in range(NQ):
            cb = run // 128
            for ww in range(sg * SGW, (sg + 1) * SGW):
                rowstart[ww, qq] = run
                run += maxcnt[ww, qq]
            run = -(-run // 128) * 128
            call_info.append((sg, qq, cb, run // 128 - cb))
        sg_tiles[sg] = run // 128 - sg_base[sg]
    TOT = run // 128
    Tsgmax = int(sg_tiles.max())

    # tile span of each (w, q): tw0..tw0+Twq-1 (boundary tiles shared)
    tw0 = rowstart // 128
    Twq = np.where(maxcnt > 0,
                   (rowstart + maxcnt - 1) // 128 - tw0 + 1, 0)

    # ul layout: window-major
    Tw = Twq.sum(axis=1)                                 # [WIN]
    Uoff = np.zeros(WIN + 1, np.int64)
    np.cumsum(Tw, out=Uoff[1:])
    Qloc = np.zeros((WIN, NQ), np.int64)
    np.cumsum(Twq[:, :-1], axis=1, out=Qloc[:, 1:])
    Twmax = int(Tw.max())

    # per-edge ranks within (core, w, q); secondary sort by table row for
    # DRAM locality of the gather descriptors
    order = np.lexsort((vloc, gk))
    gk_s = gk[order]
    starts = np.zeros(CORES * WIN * NQ + 1, np.int64)
    np.cumsum(np.bincount(gk_s, minlength=CORES * WIN * NQ), out=starts[1:])
    rank = np.arange(len(gk_s)) - starts[gk_s]
    cc_s, w_s, q_s = cc[order], w[order], q[order]
    vloc_s, col_s = vloc[order], col[order]

    rpos = rowstart[w_s, q_s] + rank                     # absolute row in G
    idx_flat = np.zeros((CORES, TOT * 128), np.int16)
    idx_flat[cc_s, rpos] = vloc_s
    ULTOT = int(Uoff[WIN])                               # > TOT: shared tiles
    ul = np.full((CORES, 128, ULTOT), -1.0, np.float32)
    ul[cc_s, rpos % 128,
       Uoff[w_s] + Qloc[w_s, q_s] + rpos // 128 - tw0[w_s, q_s]] = col_s

    # gather idx wrap: index j -> partition j%16 (replicated x8), col j//16
    idx16 = idx_flat.reshape(CORES, TOT * 8, 16).transpose(0, 2, 1)
    idx16 = np.ascontiguousarray(np.tile(idx16, (1, 8, 1)))  # [cores,128,TOT*8]

    # fp16 quartile tables, rows zero-padded to 128 elems (256B)
    xq = []
    for qq in range(NQ):
        t = np.zeros((VQ, 128), fp16)
        t[:, :D] = X[qq * VQ:(qq + 1) * VQ]
        xq.append(t)

    # own features (feature-major) + recip per slot
    xt = np.zeros((CORES, D, CAP), fp16)
    rec = np.zeros((CORES, 1, CAP), fp16)
    valid = slot_node >= 0
    for c in range(CORES):
        ids = slot_node[c][valid[c]]
        xt[c][:, :len(ids)] = X[ids].T
        rec[c][0, :len(ids)] = recip[ids]

    # per-window tile map: k-th matmul of window w reads G column gcols[w][k]
    gcols = []
    for ww in range(WIN):
        sgb = sg_base[ww // SGW]
        cols = []
        for qq in range(NQ):
            for j in range(int(Twq[ww, qq])):
                cols.append(int(tw0[ww, qq] - sgb + j))
        gcols.append(cols)

    return dict(Twq=Twq, Tw=Tw, TOT=TOT, ULTOT=ULTOT, Tsgmax=Tsgmax,
                Twmax=Twmax, Uoff=Uoff, sg_base=sg_base, sg_tiles=sg_tiles,
                call_info=call_info, gcols=gcols, idx16=idx16,
                ul=ul.astype(fp16), xq=xq, xt=xt, rec=rec,
                slot_node=slot_node)


def _build(prep):
    from concourse import bacc, mybir, tile
    f32 = mybir.dt.float32
    fp16 = mybir.dt.float16
    i16 = mybir.dt.int16
    AF = mybir.ActivationFunctionType
    ALU = mybir.AluOpType

    TOT, Tsgmax, Twmax = prep["TOT"], prep["Tsgmax"], prep["Twmax"]
    ULTOT = prep["ULTOT"]
    Tw, Uoff = prep["Tw"], prep["Uoff"]
    sg_base, sg_tiles = prep["sg_base"], prep["sg_tiles"]
    call_info, gcols = prep["call_info"], prep["gcols"]

    nc = bacc.Bacc("TRN2", debug=False, num_swdge_queues=4)

    def din(name, shape, dt=f32):
        return nc.dram_tensor(name, shape, dt, kind="ExternalInput")

    xqh = [din(f"x{qq}", [VQ, 128], fp16) for qq in range(NQ)]
    idxh = din("idx", [128, TOT * 8], i16)
    ulh = din("ul", [128, ULTOT], fp16)
    xth = din("xt", [D, CAP], fp16)
    rech = din("rec", [1, CAP], fp16)
    w1h = din("w1", [2 * D, 128], fp16)
    w2h = din("w2", [128, D], fp16)
    w3h = din("w3", [D, 67], fp16)
    p1h = din("p1", [D, 32], fp16)
    p2h = din("p2", [32, 1], fp16)
    b1h = din("b1", [128, 1])
    b2h = din("b2", [D, 1])
    b3h = din("b3", [67, 1])
    pb1h = din("pb1", [32, 1])
    pb2h = din("pb2", [1, 1])
    o67h = nc.dram_tensor("o67", [67, CAP], f32, kind="ExternalOutput")
    oph = nc.dram_tensor("op", [1, CAP], f32, kind="ExternalOutput")

    with tile.TileContext(nc) as tc:
        with (
            tc.tile_pool(name="const", bufs=1) as cpool,
            tc.tile_pool(name="gat", bufs=3) as gpool,
            tc.tile_pool(name="seg", bufs=3) as spool,
            tc.tile_pool(name="nsum", bufs=1) as npool,
            tc.tile_pool(name="mlp", bufs=2) as mpool,
            tc.tile_pool(name="pseg", bufs=2, space="PSUM") as psseg,
            tc.tile_pool(name="pmlp", bufs=2, space="PSUM") as psmlp,
        ):
            iota = cpool.tile([128, Twmax, WC], fp16)
            nc.gpsimd.iota(iota[:], pattern=[[0, Twmax], [1, WC]], base=0,
                           channel_multiplier=0,
                           allow_small_or_imprecise_dtypes=True)
            ones = cpool.tile([1, D], fp16)
            nc.vector.memset(ones[:], 1.0)

            def load_const(h, shape, dt=f32, src=None, suffix=""):
                nm = f"c_{h.name}{suffix}"
                t = cpool.tile(shape, dt, name=nm, tag=nm)
                nc.sync.dma_start(t[:], h[:] if src is None else src)
                return t

            # idx + ul first: gathers and S-builds block on these
            idx_sg = []
            for sg in range(NSG):
                a, n = int(sg_base[sg]), int(sg_tiles[sg])
                idx_sg.append(load_const(idxh, [128, n * 8], i16,
                                         src=idxh[:, a * 8:(a + n) * 8],
                                         suffix=f"_{sg}"))
            ul_all = load_const(ulh, [128, ULTOT], fp16)
            w1_t = load_const(w1h, [2 * D, 128], fp16)
            w2_t = load_const(w2h, [128, D], fp16)
            w3_t = load_const(w3h, [D, 67], fp16)
            p1_t = load_const(p1h, [D, 32], fp16)
            p2_t = load_const(p2h, [32, 1], fp16)
            b1_t = load_const(b1h, [128, 1])
            b2_t = load_const(b2h, [D, 1])
            b3_t = load_const(b3h, [67, 1])
            pb1_t = load_const(pb1h, [32, 1])
            pb2_t = load_const(pb2h, [1, 1])
            rec_all = load_const(rech, [1, CAP], fp16)

            nsum_tiles = {}
            for ci in range(NCHUNK):
                nsum_tiles[ci] = npool.tile([D, CHUNK], f32, tag=f"nsum{ci}",
                                            name=f"nsum{ci}")

            gq = 0
            for sg in range(NSG + 1):
                mlp_pending = [sg - 1] if sg else []
                if sg == NSG:
                    # final lagged chunk only; no gathers/windows
                    sgl = []
                else:
                    sgl = list(range(sg * SGW, (sg + 1) * SGW))
                # ---- gathers: one per quartile over 4 windows ----
                if sgl:
                    G = gpool.tile([128, Tsgmax, 128], fp16, tag="G")
                    sgb = int(sg_base[sg])
                for (sg_, qq, cb, nt) in (call_info if sgl else []):
                    if sg_ != sg or nt == 0:
                        continue
                    for t0 in range(0, nt, GATHER_TILES):
                        ct = min(GATHER_TILES, nt - t0)
                        lb = cb - sgb + t0
                        nc.gpsimd.dma_gather(
                            G[:, lb:lb + ct, :], xqh[qq][:],
                            idx_sg[sg][:, lb * 8:(lb + ct) * 8],
                            ct * 128, ct * 128, 128,
                            queue_num=gq % 4)
                        gq += 1

                # ---- per-window one-hot scatter-sum ----
                for ww in sgl:
                    Tww = int(Tw[ww])
                    if Tww == 0:
                        continue
                    ub = int(Uoff[ww])
                    S = spool.tile([128, Twmax, WC], fp16, tag="S")
                    nc.vector.tensor_tensor(
                        out=S[:, :Tww, :], in0=iota[:, :Tww, :],
                        in1=ul_all[:, ub:ub + Tww].broadcast_to(
                            [128, Tww, WC]),
                        op=ALU.is_equal)
                    ps = psseg.tile([D, WC], f32, tag="ps")
                    cols = gcols[ww]
                    for k, gc in enumerate(cols):
                        nc.tensor.matmul(
                            ps[:], lhsT=G[:, gc, :D], rhs=S[:, k, :],
                            start=(k == 0), stop=(k == len(cols) - 1))
                    ci, off = divmod(ww * WC, CHUNK)
                    nc.scalar.copy(nsum_tiles[ci][:, off:off + WC], ps[:])

                # ---- MLP chunk, lagged one supergroup: runs while the
                # current supergroup's gathers drain ----
                for ci in mlp_pending:
                    base = ci * CHUNK
                    cs = CHUNK
                    rb = psmlp.tile([D, cs], f32, tag="small")
                    nc.tensor.matmul(rb[:], lhsT=ones[:],
                                     rhs=rec_all[:, base:base + cs],
                                     start=True, stop=True)
                    ctx = mpool.tile([2 * D, cs], fp16, tag="ctx")
                    nc.scalar.dma_start(ctx[:D, :], xth[:, base:base + cs])
                    nc.vector.tensor_tensor(out=ctx[D:, :],
                                            in0=nsum_tiles[ci][:],
                                            in1=rb[:], op=ALU.mult)

                    h1p = psmlp.tile([128, cs], f32, tag="big")
                    nc.tensor.matmul(h1p[:], lhsT=w1_t[:], rhs=ctx[:],
                                     start=True, stop=True)
                    h1 = mpool.tile([128, cs], fp16, tag="h1")
                    nc.scalar.activation(out=h1[:], in_=h1p[:], func=AF.Relu,
                                         bias=b1_t[:], scale=1.0)

                    h2p = psmlp.tile([D, cs], f32, tag="big")
                    nc.tensor.matmul(h2p[:], lhsT=w2_t[:], rhs=h1[:],
                                     start=True, stop=True)
                    h2 = mpool.tile([D, cs], fp16, tag="h2")
                    nc.scalar.activation(out=h2[:], in_=h2p[:], func=AF.Relu,
                                         bias=b2_t[:], scale=1.0)

                    gp = psmlp.tile([67, cs], f32, tag="big")
                    nc.tensor.matmul(gp[:], lhsT=w3_t[:], rhs=h2[:],
                                     start=True, stop=True)
                    g67 = mpool.tile([67, cs], f32, tag="g67")
                    nc.scalar.activation(out=g67[:], in_=gp[:],
                                         func=AF.Identity, bias=b3_t[:],
                                         scale=1.0)
                    nc.scalar.dma_start(o67h[:, base:base + cs], g67[:])
                    g64b = mpool.tile([D, cs], fp16, tag="g64b")
                    nc.scalar.copy(g64b[:], g67[:D, :])

                    pp = psmlp.tile([32, cs], f32, tag="small")
                    nc.tensor.matmul(pp[:], lhsT=p1_t[:], rhs=g64b[:],
                                     start=True, stop=True)
                    pa = mpool.tile([32, cs], fp16, tag="pa")
                    nc.scalar.activation(out=pa[:], in_=pp[:], func=AF.Relu,
                                         bias=pb1_t[:], scale=1.0)

                    prp = psmlp.tile([1, cs], f32, tag="small")
                    nc.tensor.matmul(prp[:], lhsT=p2_t[:], rhs=pa[:],
                                     start=True, stop=True)
                    pr = mpool.tile([1, cs], f32, tag="pr")
                    nc.scalar.activation(out=pr[:], in_=prp[:],
                                         func=AF.Sigmoid, bias=pb2_t[:],
                                         scale=1.0)
                    nc.scalar.dma_start(oph[:, base:base + cs], pr[:])

    nc.compile()
    return nc


def _in_maps(prep, W1, b1, W2, b2, W3, b3, P1, pb1, P2, pb2):
    fp16 = np.float16
    W3 = np.asarray(W3, np.float32)
    b3 = np.asarray(b3, np.float32)
    w3p = np.ascontiguousarray(np.concatenate([W3[:, 3:], W3[:, :3]], axis=1))
    b3p = np.concatenate([b3[3:], b3[:3]])
    shared = {
        "w1": np.asarray(W1, np.float32).astype(fp16),
        "w2": np.asarray(W2, np.float32).astype(fp16),
        "w3": w3p.astype(fp16),
        "p1": np.asarray(P1, np.float32).astype(fp16),
        "p2": np.asarray(P2, np.float32).astype(fp16),
        "b1": np.asarray(b1, np.float32).reshape(-1, 1),
        "b2": np.asarray(b2, np.float32).reshape(-1, 1),
        "b3": b3p.astype(np.float32).reshape(-1, 1),
        "pb1": np.asarray(pb1, np.float32).reshape(-1, 1),
        "pb2": np.asarray(pb2, np.float32).reshape(-1, 1),
    }
    for qq, t in enumerate(prep["xq"]):
        shared[f"x{qq}"] = t
    maps = []
    for c in range(CORES):
        m = dict(shared)
        m["idx"] = prep["idx16"][c]
        m["ul"] = prep["ul"][c]
        m["xt"] = prep["xt"][c]
        m["rec"] = prep["rec"][c]
        maps.append(m)
    return maps


def _assemble(results, prep):
    slot_node = prep["slot_node"]
    out = np.zeros((N, D + 4), np.float32)
    for c, r in enumerate(results):
        ids = slot_node[c]
        valid = ids >= 0
        ids = ids[valid]
        o67 = r["o67"]
        op = r["op"]
        sel = np.where(valid)[0]
        out[ids, 0:3] = o67[D:D + 3, sel].T
        out[ids, 3:3 + D] = o67[:D, sel].T
        out[ids, 3 + D] = op[0, sel]
    return out


def kernel(**inputs):
    from concourse.bass_utils import run_bass_kernel_spmd
    prep = _host_prep(inputs["node_features"], inputs["node_operations"],
                      inputs["edge_index"])
    nc = _build(prep)
    maps = _in_maps(prep, inputs["W1"], inputs["b1"], inputs["W2"],
                    inputs["b2"], inputs["W3"], inputs["b3"], inputs["P1"],
                    inputs["pb1"], inputs["P2"], inputs["pb2"])
    res = run_bass_kernel_spmd(nc, maps, core_ids=list(range(CORES)))
    return _assemble(res.results, prep)


# revision 30
# speedup vs baseline: 1.1890x; 1.1890x over previous
"""Trainium2 Bass kernel for nn_NodeGenerator (GNN message passing).

Strategy (8 NeuronCores, SPMD, no collectives):
  - Only candidate nodes (softmax class-0 > 0.5 and deg > 0, ~12% of N)
    produce nonzero output rows. Host computes the mask (f64) and
    COMPACTS: candidates are dealt round-robin (by descending degree)
    to the 8 cores, ~1508/core, padded to 24 windows of 64 owners.
  - The full node-feature table is replicated per core as 4 quartile
    tables of fp16 rows zero-padded to 128 elems (256 B — the dma_gather
    minimum), so int16 indices can address any row and gathered rows
    feed the PE directly with no convert pass.
  - Directed edges of each core's owners are grouped per
    (supergroup of 4 windows, quartile) and fetched with ONE dma_gather
    per group (24 calls/core, ~2.3K rows each; ring cost is
    num_idxs/16+1 so large calls fit the default SWDGE ring).
  - Per 64-owner window: one-hot S [128, T, 64] built on DVE
    (iota-compare vs per-row owner column), then T accumulating fp16
    PE matmuls G_tile.T @ S_tile give feature-major neighbor sums
    [64, 64] in fp32 PSUM.
  - Neighbor mean + MLP run feature-major over 3 chunks of 512
    candidate columns; recip is broadcast via a rank-1 matmul. No mask
    multiply on device: every compacted column is a candidate, and pad
    columns are simply never scattered back on host.
"""

import numpy as np

N = 100000
D = 64
CORES = 8
NQ = 4
VQ = 25000          # rows per quartile table
WC = 64             # owners per window
WIN = 28            # windows per core
CAP = WIN * WC      # 1536 candidate slots per core
SGW = 4             # windows per gather supergroup
NSG = WIN // SGW    # supergroups
CHUNK = SGW * WC    # MLP column tile = one supergroup (256)
NCHUNK = CAP // CHUNK
GATHER_TILES = 8    # tiles (128 rows each) per dma_gather call: the SWDGE
                    # ring holds 1024 row-descriptors (hard cap per call),
                    # and per-call DMA-engine setup (~0.45us) favors max size


def _host_prep(node_features, node_operations, edge_index):
    fp16 = np.float16
    X = np.ascontiguousarray(np.asarray(node_features, dtype=np.float32))
    ops = np.asarray(node_operations, dtype=np.float32)
    ei = np.asarray(edge_index, dtype=np.int64)
    src, dst = ei[0], ei[1]
    U = np.concatenate([src, dst])
    V = np.concatenate([dst, src])

    deg = np.bincount(U, minlength=N).astype(np.int64)
    o = ops.astype(np.float64)
    e = np.exp(o - o.max(axis=1, keepdims=True))
    p0 = e[:, 0] / e.sum(axis=1)
    maskf = (p0 > 0.5) & (deg > 0)
    recip = (1.0 / np.maximum(deg, 1.0)).astype(np.float32)

    # Compact: deal candidates (desc degree) to cores; each consecutive
    # 8-group lands on one slot position. Within a group the core
    # permutation is chosen to balance per-(window, quartile) loads, which
    # sets the shared (max-over-core) gather row caps.
    cand = np.where(maskf)[0]
    cand = cand[np.argsort(-deg[cand], kind="stable")]
    ncand = len(cand)
    candidx = np.full(N, -1, np.int64)
    candidx[cand] = np.arange(ncand)

    keepe = maskf[U]
    Uk, Vk = U[keepe], V[keepe]
    qe = Vk // VQ
    qv = np.zeros((ncand, NQ), np.int64)
    np.add.at(qv, (candidx[Uk], qe), 1)

    # positions interleave round-robin over windows so every window gets
    # ~equal candidate count and a mix of degrees
    gidx = np.arange(ncand) // CORES
    pos_of_c = (gidx % WIN) * WC + gidx // WIN
    assert (gidx // WIN).max() < WC, (ncand, CAP)

    core_of_c = np.empty(ncand, np.int64)
    loads = np.zeros((WIN, CORES, NQ), np.int64)
    for i0 in range(0, ncand, CORES):
        wload = loads[(i0 // CORES) % WIN]
        n = min(CORES, ncand - i0)
        freec = list(range(CORES))
        for i in range(i0, i0 + n):
            cost = ((wload[freec] + qv[i]) ** 2).sum(axis=1)
            j = int(np.argmin(cost))
            c = freec.pop(j)
            core_of_c[i] = c
            wload[c] += qv[i]

    core_of_node = np.full(N, -1, np.int64)
    pos_of_node = np.full(N, -1, np.int64)
    core_of_node[cand] = core_of_c
    pos_of_node[cand] = pos_of_c

    # node id per (core, slot) for output scatter
    slot_node = np.full((CORES, CAP), -1, np.int64)
    slot_node[core_of_c, pos_of_c] = cand

    # Edges owned by candidates
    keep = maskf[U]
    Uk, Vk = U[keep], V[keep]
    cc = core_of_node[Uk]
    pp = pos_of_node[Uk]
    w = pp // WC
    col = (pp % WC).astype(np.float32)
    q = Vk // VQ
    vloc = (Vk - q * VQ).astype(np.int16)

    # counts per (core, w, q); shared row caps = max over cores. Windows are
    # packed back-to-back (no per-window tile alignment) inside each
    # (supergroup, quartile) block; only block ends align to 128.
    gk = (cc * WIN + w) * NQ + q
    cnt = np.bincount(gk, minlength=CORES * WIN * NQ).reshape(CORES, WIN, NQ)
    maxcnt = cnt.max(axis=0)                             # [WIN, NQ] rows

    rowstart = np.zeros((WIN, NQ), np.int64)
    run = 0                                              # rows
    sg_base = np.zeros(NSG, np.int64)                    # tiles
    sg_tiles = np.zeros(NSG, np.int64)
    call_info = []   # (sg, q, tile_base_global, ntiles)
    for sg in range(NSG):
        sg_base[sg] = run // 128
        for qq in range(NQ):
            cb = run // 128
            for ww in range(sg * SGW, (sg + 1) * SGW):
                rowstart[ww, qq] = run
                run += maxcnt[ww, qq]
            run = -(-run // 128) * 128
            call_info.append((sg, qq, cb, run // 128 - cb))
        sg_tiles[sg] = run // 128 - sg_base[sg]
    TOT = run // 128
    Tsgmax = int(sg_tiles.max())

    # tile span of each (w, q): tw0..tw0+Twq-1 (boundary tiles shared)
    tw0 = rowstart // 128
    Twq = np.where(maxcnt > 0,
                   (rowstart + maxcnt - 1) // 128 - tw0 + 1, 0)

    # ul layout: window-major
    Tw = Twq.sum(axis=1)                                 # [WIN]
    Uoff = np.zeros(WIN + 1, np.int64)
    np.cumsum(Tw, out=Uoff[1:])
    Qloc = np.zeros((WIN, NQ), np.int64)
    np.cumsum(Twq[:, :-1], axis=1, out=Qloc[:, 1:])
    Twmax = int(Tw.max())

    # per-edge ranks within (core, w, q); secondary sort by table row for
    # DRAM locality of the gather descriptors
    order = np.lexsort((vloc, gk))
    gk_s = gk[order]
    starts = np.zeros(CORES * WIN * NQ + 1, np.int64)
    np.cumsum(np.bincount(gk_s, minlength=CORES * WIN * NQ), out=starts[1:])
    rank = np.arange(len(gk_s)) - starts[gk_s]
    cc_s, w_s, q_s = cc[order], w[order], q[order]
    vloc_s, col_s = vloc[order], col[order]

    rpos = rowstart[w_s, q_s] + rank                     # absolute row in G
    idx_flat = np.zeros((CORES, TOT * 128), np.int16)
    idx_flat[cc_s, rpos] = vloc_s
    ULTOT = int(Uoff[WIN])                               # > TOT: shared tiles
    ul = np.full((CORES, 128, ULTOT), -1.0, np.float32)
    ul[cc_s, rpos % 128,
       Uoff[w_s] + Qloc[w_s, q_s] + rpos // 128 - tw0[w_s, q_s]] = col_s

    # gather idx wrap: index j -> partition j%16 (replicated x8), col j//16
    idx16 = idx_flat.reshape(CORES, TOT * 8, 16).transpose(0, 2, 1)
    idx16 = np.ascontiguousarray(np.tile(idx16, (1, 8, 1)))  # [cores,128,TOT*8]

    # fp16 quartile tables, rows zero-padded to 128 elems (256B)
    xq = []
    for qq in range(NQ):
        t = np.zeros((VQ, 128), fp16)
        t[:, :D] = X[qq * VQ:(qq + 1) * VQ]
        xq.append(t)

    # own features (feature-major) + recip per slot
    xt = np.zeros((CORES, D, CAP), fp16)
    rec = np.zeros((CORES, 1, CAP), fp16)
    valid = slot_node >= 0
    for c in range(CORES):
        sel = np.where(valid[c])[0]
        ids = slot_node[c][sel]
        xt[c][:, sel] = X[ids].T
        rec[c][0, sel] = recip[ids]

    # per-window tile map: k-th matmul of window w reads G column gcols[w][k]
    gcols = []
    for ww in range(WIN):
        sgb = sg_base[ww // SGW]
        cols = []
        for qq in range(NQ):
            for j in range(int(Twq[ww, qq])):
                cols.append(int(tw0[ww, qq] - sgb + j))
        gcols.append(cols)

    return dict(Twq=Twq, Tw=Tw, TOT=TOT, ULTOT=ULTOT, Tsgmax=Tsgmax,
                Twmax=Twmax, Uoff=Uoff, sg_base=sg_base, sg_tiles=sg_tiles,
                call_info=call_info, gcols=gcols, idx16=idx16,
                ul=ul.astype(fp16), xq=xq, xt=xt, rec=rec,
                slot_node=slot_node)


def _build(prep):
    from concourse import bacc, mybir, tile
    f32 = mybir.dt.float32
    fp16 = mybir.dt.float16
    i16 = mybir.dt.int16
    AF = mybir.ActivationFunctionType
    ALU = mybir.AluOpType

    TOT, Tsgmax, Twmax = prep["TOT"], prep["Tsgmax"], prep["Twmax"]
    ULTOT = prep["ULTOT"]
    Tw, Uoff = prep["Tw"], prep["Uoff"]
    sg_base, sg_tiles = prep["sg_base"], prep["sg_tiles"]
    call_info, gcols = prep["call_info"], prep["gcols"]

    nc = bacc.Bacc("TRN2", debug=False, num_swdge_queues=4)

    def din(name, shape, dt=f32):
        return nc.dram_tensor(name, shape, dt, kind="ExternalInput")

    xqh = [din(f"x{qq}", [VQ, 128], fp16) for qq in range(NQ)]
    idxh = din("idx", [128, TOT * 8], i16)
    ulh = din("ul", [128, ULTOT], fp16)
    xth = din("xt", [D, CAP], fp16)
    rech = din("rec", [1, CAP], fp16)
    w1h = din("w1", [2 * D, 128], fp16)
    w2h = din("w2", [128, D], fp16)
    w3h = din("w3", [D, 67], fp16)
    p1h = din("p1", [D, 32], fp16)
    p2h = din("p2", [32, 1], fp16)
    b1h = din("b1", [128, 1])
    b2h = din("b2", [D, 1])
    b3h = din("b3", [67, 1])
    pb1h = din("pb1", [32, 1])
    pb2h = din("pb2", [1, 1])
    o67h = nc.dram_tensor("o67", [67, CAP], f32, kind="ExternalOutput")
    oph = nc.dram_tensor("op", [1, CAP], f32, kind="ExternalOutput")

    with tile.TileContext(nc) as tc:
        with (
            tc.tile_pool(name="const", bufs=1) as cpool,
            tc.tile_pool(name="gat", bufs=3) as gpool,
            tc.tile_pool(name="seg", bufs=3) as spool,
            tc.tile_pool(name="nsum", bufs=1) as npool,
            tc.tile_pool(name="mlp", bufs=2) as mpool,
            tc.tile_pool(name="pseg", bufs=2, space="PSUM") as psseg,
            tc.tile_pool(name="pmlp", bufs=2, space="PSUM") as psmlp,
        ):
            iota = cpool.tile([128, Twmax, WC], fp16)
            nc.gpsimd.iota(iota[:], pattern=[[0, Twmax], [1, WC]], base=0,
                           channel_multiplier=0,
                           allow_small_or_imprecise_dtypes=True)
            ones = cpool.tile([1, D], fp16)
            nc.vector.memset(ones[:], 1.0)

            def load_const(h, shape, dt=f32, src=None, suffix=""):
                nm = f"c_{h.name}{suffix}"
                t = cpool.tile(shape, dt, name=nm, tag=nm)
                nc.sync.dma_start(t[:], h[:] if src is None else src)
                return t

            # idx + ul first: gathers and S-builds block on these
            idx_sg = []
            for sg in range(NSG):
                a, n = int(sg_base[sg]), int(sg_tiles[sg])
                idx_sg.append(load_const(idxh, [128, n * 8], i16,
                                         src=idxh[:, a * 8:(a + n) * 8],
                                         suffix=f"_{sg}"))
            ul_all = load_const(ulh, [128, ULTOT], fp16)
            w1_t = load_const(w1h, [2 * D, 128], fp16)
            w2_t = load_const(w2h, [128, D], fp16)
            w3_t = load_const(w3h, [D, 67], fp16)
            p1_t = load_const(p1h, [D, 32], fp16)
            p2_t = load_const(p2h, [32, 1], fp16)
            b1_t = load_const(b1h, [128, 1])
            b2_t = load_const(b2h, [D, 1])
            b3_t = load_const(b3h, [67, 1])
            pb1_t = load_const(pb1h, [32, 1])
            pb2_t = load_const(pb2h, [1, 1])
            rec_all = load_const(rech, [1, CAP], fp16)

            nsum_tiles = {}
            for ci in range(NCHUNK):
                nsum_tiles[ci] = npool.tile([D, CHUNK], f32, tag=f"nsum{ci}",
                                            name=f"nsum{ci}")

            gq = 0
            for sg in range(NSG + 1):
                mlp_pending = [sg - 1] if sg else []
                if sg == NSG:
                    # final lagged chunk only; no gathers/windows
                    sgl = []
                else:
                    sgl = list(range(sg * SGW, (sg + 1) * SGW))
                # ---- gathers: one per quartile over 4 windows ----
                if sgl:
                    G = gpool.tile([128, Tsgmax, 128], fp16, tag="G")
                    sgb = int(sg_base[sg])
                for (sg_, qq, cb, nt) in (call_info if sgl else []):
                    if sg_ != sg or nt == 0:
                        continue
                    for t0 in range(0, nt, GATHER_TILES):
                        ct = min(GATHER_TILES, nt - t0)
                        lb = cb - sgb + t0
                        nc.gpsimd.dma_gather(
                            G[:, lb:lb + ct, :], xqh[qq][:],
                            idx_sg[sg][:, lb * 8:(lb + ct) * 8],
                            ct * 128, ct * 128, 128,
                            queue_num=gq % 4)
                        gq += 1

                # ---- per-window one-hot scatter-sum ----
                for ww in sgl:
                    Tww = int(Tw[ww])
                    if Tww == 0:
                        continue
                    ub = int(Uoff[ww])
                    S = spool.tile([128, Twmax, WC], fp16, tag="S")
                    nc.vector.tensor_tensor(
                        out=S[:, :Tww, :], in0=iota[:, :Tww, :],
                        in1=ul_all[:, ub:ub + Tww].broadcast_to(
                            [128, Tww, WC]),
                        op=ALU.is_equal)
                    ps = psseg.tile([D, WC], f32, tag="ps")
                    cols = gcols[ww]
                    for k, gc in enumerate(cols):
                        nc.tensor.matmul(
                            ps[:], lhsT=G[:, gc, :D], rhs=S[:, k, :],
                            start=(k == 0), stop=(k == len(cols) - 1))
                    ci, off = divmod(ww * WC, CHUNK)
                    nc.scalar.copy(nsum_tiles[ci][:, off:off + WC], ps[:])

                # ---- MLP chunk, lagged one supergroup: runs while the
                # current supergroup's gathers drain ----
                for ci in mlp_pending:
                    base = ci * CHUNK
                    cs = CHUNK
                    rb = psmlp.tile([D, cs], f32, tag="small")
                    nc.tensor.matmul(rb[:], lhsT=ones[:],
                                     rhs=rec_all[:, base:base + cs],
                                     start=True, stop=True)
                    ctx = mpool.tile([2 * D, cs], fp16, tag="ctx")
                    nc.sync.dma_start(ctx[:D, :], xth[:, base:base + cs])
                    nc.vector.tensor_tensor(out=ctx[D:, :],
                                            in0=nsum_tiles[ci][:],
                                            in1=rb[:], op=ALU.mult)

                    h1p = psmlp.tile([128, cs], f32, tag="big")
                    nc.tensor.matmul(h1p[:], lhsT=w1_t[:], rhs=ctx[:],
                                     start=True, stop=True)
                    h1 = mpool.tile([128, cs], fp16, tag="h1")
                    nc.scalar.activation(out=h1[:], in_=h1p[:], func=AF.Relu,
                                         bias=b1_t[:], scale=1.0)

                    h2p = psmlp.tile([D, cs], f32, tag="big")
                    nc.tensor.matmul(h2p[:], lhsT=w2_t[:], rhs=h1[:],
                                     start=True, stop=True)
                    h2 = mpool.tile([D, cs], fp16, tag="h2")
                    nc.scalar.activation(out=h2[:], in_=h2p[:], func=AF.Relu,
                                         bias=b2_t[:], scale=1.0)

                    gp = psmlp.tile([67, cs], f32, tag="big")
                    nc.tensor.matmul(gp[:], lhsT=w3_t[:], rhs=h2[:],
                                     start=True, stop=True)
                    g67 = mpool.tile([67, cs], f32, tag="g67")
                    nc.scalar.activation(out=g67[:], in_=gp[:],
                                         func=AF.Identity, bias=b3_t[:],
                                         scale=1.0)
                    nc.sync.dma_start(o67h[:, base:base + cs], g67[:])
                    g64b = mpool.tile([D, cs], fp16, tag="g64b")
                    nc.scalar.copy(g64b[:], g67[:D, :])

                    pp = psmlp.tile([32, cs], f32, tag="small")
                    nc.tensor.matmul(pp[:], lhsT=p1_t[:], rhs=g64b[:],
                                     start=True, stop=True)
                    pa = mpool.tile([32, cs], fp16, tag="pa")
                    nc.scalar.activation(out=pa[:], in_=pp[:], func=AF.Relu,
                                         bias=pb1_t[:], scale=1.0)

                    prp = psmlp.tile([1, cs], f32, tag="small")
                    nc.tensor.matmul(prp[:], lhsT=p2_t[:], rhs=pa[:],
                                     start=True, stop=True)
                    pr = mpool.tile([1, cs], f32, tag="pr")
                    nc.scalar.activation(out=pr[:], in_=prp[:],
                                         func=AF.Sigmoid, bias=pb2_t[:],
                                         scale=1.0)
                    nc.sync.dma_start(oph[:, base:base + cs], pr[:])

    nc.compile()
    return nc


def _in_maps(prep, W1, b1, W2, b2, W3, b3, P1, pb1, P2, pb2):
    fp16 = np.float16
    W3 = np.asarray(W3, np.float32)
    b3 = np.asarray(b3, np.float32)
    w3p = np.ascontiguousarray(np.concatenate([W3[:, 3:], W3[:, :3]], axis=1))
    b3p = np.concatenate([b3[3:], b3[:3]])
    shared = {
        "w1": np.asarray(W1, np.float32).astype(fp16),
        "w2": np.asarray(W2, np.float32).astype(fp16),
        "w3": w3p.astype(fp16),
        "p1": np.asarray(P1, np.float32).astype(fp16),
        "p2": np.asarray(P2, np.float32).astype(fp16),
        "b1": np.asarray(b1, np.float32).reshape(-1, 1),
        "b2": np.asarray(b2, np.float32).reshape(-1, 1),
        "b3": b3p.astype(np.float32).reshape(-1, 1),
        "pb1": np.asarray(pb1, np.float32).reshape(-1, 1),
        "pb2": np.asarray(pb2, np.float32).reshape(-1, 1),
    }
    for qq, t in enumerate(prep["xq"]):
        shared[f"x{qq}"] = t
    maps = []
    for c in range(CORES):
        m = dict(shared)
        m["idx"] = prep["idx16"][c]
        m["ul"] = prep["ul"][c]
        m["xt"] = prep["xt"][c]
        m["rec"] = prep["rec"][c]
        maps.append(m)
    return maps


def _assemble(results, prep):
    slot_node = prep["slot_node"]
    out = np.zeros((N, D + 4), np.float32)
    for c, r in enumerate(results):
        ids = slot_node[c]
        valid = ids >= 0
        ids = ids[valid]
        o67 = r["o67"]
        op = r["op"]
        sel = np.where(valid)[0]
        out[ids, 0:3] = o67[D:D + 3, sel].T
        out[ids, 3:3 + D] = o67[:D, sel].T
        out[ids, 3 + D] = op[0, sel]
    return out


def kernel(**inputs):
    from concourse.bass_utils import run_bass_kernel_spmd
    prep = _host_prep(inputs["node_features"], inputs["node_operations"],
                      inputs["edge_index"])
    nc = _build(prep)
    maps = _in_maps(prep, inputs["W1"], inputs["b1"], inputs["W2"],
                    inputs["b2"], inputs["W3"], inputs["b3"], inputs["P1"],
                    inputs["pb1"], inputs["P2"], inputs["pb2"])
    res = run_bass_kernel_spmd(nc, maps, core_ids=list(range(CORES)))
    return _assemble(res.results, prep)
